# revision 20
# baseline (speedup 1.0000x reference)
"""GAT kernel for Trainium2 (Bass/Tile), data-parallel over batch on 8 cores.

Per-core math (one batch element, N=1024 nodes, H=4 heads, D=E=128). The
softmax numerator exp(lrelu(z_ij)) with z_ij = a_s_i + a_n_j is rewritten
using monotonicity of exp:

  exp(lrelu(z))/s_i = max(w_i, r_j) * v_j       (lrelu = max(z, 0.2 z))
  w = exp(0.8 a_s),  r = exp(-0.8 a_n),  v = exp(a_n - 2),
  s_i = exp(0.2 a_s_i) cancels in the softmax.

a_s/a_n are O(N*H) row vectors; they and their exp transforms are computed
on the host and DMA'd in (~24 KB per core), so the device touches only the
O(N^2) work.  On-core per head:
  p[j,i] = max(w_i, r_j) * adjT[j,i]     (DVE 1-op ts-max + tt-mult; the tt
                                          of 2-3 chunks/head runs on the
                                          otherwise idle GPSIMD)
  num/den accumulate via PE matmuls against [v*feat | v]; v is folded into
  the ACT PSUM->SBUF feature copy via its per-partition scale operand.
PSUM accumulators are bank-packed [P,3,EA]: only the first matmul into a
bank uses start=True (the flag clears has_written for the whole bank), so
eight i-block groups fit in 3 banks and reciprocals batch 3 i-blocks/op.
DMA dispatches are spread across the SP/ACT/GPSIMD queues (each dma_start
occupies its queue ~0.6us) and w-rows ship as one [1,4N] partition-0 strip
so the w16 broadcast (PE ones-outer + DVE/ACT copies) finishes by ~4us."""

import os
import sys

sys.path.insert(0, "/opt/trn_rl_repo")

import numpy as np

import concourse.bass as bass
import concourse.bacc as bacc
import concourse.mybir as mybir
import concourse.tile as tile
from concourse.bass_utils import run_bass_kernel_spmd

F32 = mybir.dt.float32
F16 = mybir.dt.float16
P = 128


def build_core_program(N, H, D=128, E=128):
    """Trace the Bass program computing one batch element of the GAT."""
    nc = bacc.Bacc("TRN2", debug=False, target_bir_lowering=False)
    NCH = N // P          # node chunks
    EA = E + 1            # feat columns + v column (den)
    HP = H // 2
    SEG = 512
    segs = [(s, min(SEG, N - s)) for s in range(0, N, SEG)]
    # i-block banks: groups of <=BSZ share one PSUM bank
    BSZ = int(os.environ.get("GAT_BANKSZ", "3"))
    DEFER = int(os.environ.get("GAT_DEFER", "1"))
    banks = [list(range(b, min(b + BSZ, NCH))) for b in range(0, NCH, BSZ)]

    XOFF = H * E
    wx = nc.dram_tensor("wx", [D, XOFF + N], F16, kind="ExternalInput").ap()
    # w rows packed [1, H*N] (partition-0 strip)
    w4 = nc.dram_tensor("w4", [1, H * N], F16, kind="ExternalInput").ap()
    # rv pre-transposed on host: [P, NCH * 2H]; cols c*2H+h = r_h, c*2H+H+h = v_h
    rvT = nc.dram_tensor("rvT", [P, NCH * 2 * H], F32,
                         kind="ExternalInput").ap()
    adjT = nc.dram_tensor("adjT", [N, N], F16, kind="ExternalInput").ap()
    out = nc.dram_tensor("out", [N, H * E], F16, kind="ExternalOutput").ap()

    # gpsimd tt chunks per head (late chunks; none for the last head's tail)
    GPS_DEF = ",".join([""] * H)
    gps_spec = os.environ.get("GAT_GPSTT", GPS_DEF).split(",")
    GPS_CHUNKS = [set(int(ch) for ch in s) if s else set()
                  for s in (gps_spec + [""] * H)[:H]]
    XSEG = min(int(os.environ.get("GAT_XSEG", "256")), N)
    ASEG0 = min(int(os.environ.get("GAT_ASEG0", "256")), N)
    ASEG = min(int(os.environ.get("GAT_ASEG", "512")), N)

    with tile.TileContext(nc) as tc:
        with (
            tc.tile_pool(name="const", bufs=1) as const_pool,
            tc.tile_pool(name="xt", bufs=1) as xt_pool,
            tc.tile_pool(name="adj", bufs=1) as adj_pool,
            tc.tile_pool(name="fr", bufs=1) as fr_pool,
            tc.tile_pool(name="rv", bufs=1) as rv_pool,
            tc.tile_pool(name="wr", bufs=1) as wr_pool,
            tc.tile_pool(name="w16", bufs=1) as w16_pool,
        ):
            ones_sb = const_pool.tile([1, P], F16, tag="ones", padded_shape=[1, 256])
            nc.vector.memset(ones_sb[:], 1.0)

            kaug_t = xt_pool.tile([D, H * E], F16, tag="kaug")
            NXT = max(N // 512, 1)
            xt_t = [xt_pool.tile([D, min(512, N)], F16, tag=f"xt{i}",
                                 name=f"xt{i}") for i in range(NXT)]
            w4flat = wr_pool.tile([1, H * N], F16, tag="w4flat")
            RVPAD = ((NCH * 2 * H + 127) // 128) * 128
            rv2f = rv_pool.tile([P, NCH * 2 * H], F32, tag="rv2",
                                padded_shape=[P, RVPAD])

            def rv_ap(c, idx):
                return rv2f[:, c * 2 * H + idx:c * 2 * H + idx + 1]

            def xt_cols(s0, w):
                ti, off = s0 // 512, s0 % 512
                assert off + w <= 512
                return xt_t[ti][:, off:off + w]

            QC = 4 if NCH % 4 == 0 else 1
            NQ = NCH // QC
            adj_sb = [adj_pool.tile([P, QC * N], F16, tag=f"adj{c4}",
                                    name=f"adj{c4}") for c4 in range(NQ)]

            def adj_ap(c):
                c4, kq = c // QC, c % QC
                return adj_sb[c4][:, kq * N:(kq + 1) * N]

            def adj_dma(eng, c, aseg):
                for s in range(0, N, aseg):
                    eng.dma_start(out=adj_ap(c)[:, s:s + aseg],
                                  in_=adjT[c * P:(c + 1) * P, s:s + aseg])

            with tc.high_priority():
                # scalar (HWDGE): w rows + rv first (gate the mask) -- on
                # their own queue so no DMA-semaphore reuse delays them
                nc.scalar.dma_start(out=w4flat[:], in_=w4)
                nc.scalar.dma_start(out=rv2f[:], in_=rvT)
                adj_dma(nc.sync, 0, ASEG0)
                if NCH > 2:
                    adj_dma(nc.sync, 2, ASEG)
                for s in range(0, H * E, 256):
                    nc.sync.dma_start(out=kaug_t[:, s:s + 256],
                                      in_=wx[:, s:s + 256])
                if NCH > 4:
                    adj_dma(nc.sync, 4, ASEG)
                xslices = ([128, 128, 256, 256, 256] if N == 1024
                           else [XSEG] * (N // XSEG))
                xq = []
                s = 0
                for sw in xslices:
                    ti, off = s // 512, s % 512
                    xq.append((xt_t[ti], off, sw, s))
                    s += sw
                for i, (t, off, sw, s0) in enumerate(xq):
                    nc.sync.dma_start(out=t[:, off:off + sw],
                                      in_=wx[:, XOFF + s0:XOFF + s0 + sw])
                    if i == 1 and NCH > 6:
                        adj_dma(nc.sync, 6, ASEG)
                # scalar (HWDGE): odd adj chunks (its copies start later)
                for c in range(1, NCH, 2):
                    adj_dma(nc.scalar, c, ASEG)

            w16 = [w16_pool.tile([P, N], F16, tag=f"w16_{h}", name=f"w16_{h}")
                   for h in range(H)]
            FRPAD = ((H * EA + 255) // 256) * 256
            feat2v = [fr_pool.tile([P, H * EA], F16, tag=f"fr{c}",
                                   name=f"fr{c}", padded_shape=[P, FRPAD])
                      for c in range(NCH)]

            with (
                tc.tile_pool(name="acc_e", bufs=1, space="PSUM") as acc_e,
                tc.tile_pool(name="acc_o", bufs=1, space="PSUM") as acc_o,
                tc.tile_pool(name="proj_ps", bufs=2, space="PSUM") as proj_ps,
            ):
                # ---- w16 broadcast: gpsimd partition_broadcast for head 0
                # only (earliest need; gpsimd then stays off the SBUF port).
                # Heads 1+ go PE ones-outer -> ACT copy via borrowed acc_o
                # banks.
                bc_tags = [f"b{i}" for i in range(len(banks))]
                with tc.high_priority():
                    nc.gpsimd.partition_broadcast(
                        w16[0][:], w4flat[0:1, 0:N])
                bc_i = 0
                for h in range(1, H):
                    for s0, sw in segs:
                        tg = bc_tags[bc_i % len(bc_tags)]
                        bc_i += 1
                        bc = acc_o.tile([P, max(SEG, BSZ * EA)], F32, tag=tg,
                                        name=f"bc{h}_{s0}")
                        nc.tensor.matmul(
                            bc[:, 0:sw], ones_sb[:],
                            w4flat[:, h * N + s0:h * N + s0 + sw],
                            start=True, stop=True)
                        nc.scalar.copy(w16[h][:, s0:s0 + sw], bc[:, 0:sw])

                # ---- projection + v-scaled feature copies + v columns
                for c in range(NCH):
                    f3 = feat2v[c][:].rearrange("p (h f) -> p h f", h=H)
                    # den columns are plain ones (v lives in the ts output)
                    nc.vector.memset(f3[:, :, E:E + 1].squeeze(2), 1.0)
                    for hp in range(HP):
                        ps = proj_ps.tile([P, 2 * E], F32, tag="proj",
                                          name=f"proj{hp}_{c}")
                        nc.tensor.matmul(
                            ps[:], xt_cols(c * P, P),
                            kaug_t[:, hp * 2 * E:(hp + 1) * 2 * E],
                            start=True, stop=True)
                        ps3 = ps[:].rearrange("p (k f) -> p k f", k=2)
                        nc.scalar.copy(f3[:, 2 * hp:2 * hp + 2, 0:E], ps3[:])

                with (
                    tc.tile_pool(name="q", bufs=2) as q_pool,
                    tc.tile_pool(name="ep", bufs=2) as ep_pool,
                ):
                    pending = []

                    def emit_epis(k):
                        for _ in range(k):
                            if pending:
                                pending.pop(0)()

                    for h in range(H):
                        acc_pool = acc_e if h % 2 == 0 else acc_o
                        tags = (["A", "B", "C"] if h % 2 == 0 else bc_tags)
                        accs = [acc_pool.tile([P, len(bk), EA], F32,
                                              tag=tags[b % len(tags)],
                                              name=f"acc{h}_{b}")
                                for b, bk in enumerate(banks)]

                        def acc_ap(ib):
                            b, i = divmod(ib, BSZ)
                            return accs[b][:, i, :]

                        # masks: all ts first (adj-tolerant), then tts
                        q_t = [q_pool.tile([P, N], F16, tag=f"q{c}",
                                           name=f"q{h}_{c}")
                               for c in range(NCH)]
                        p_t = [q_pool.tile([P, N], F16, tag=f"p{c}",
                                           name=f"p{h}_{c}")
                               for c in range(NCH)]
                        ILV = int(os.environ.get("GAT_ILV", "0"))
                        if ILV:
                            for c in range(NCH):
                                nc.vector.tensor_scalar(
                                    q_t[c][:], w16[h][:], rv_ap(c, h),
                                    rv_ap(c, H + h),
                                    mybir.AluOpType.max, mybir.AluOpType.mult)
                                nc.vector.tensor_tensor(
                                    p_t[c][:], q_t[c][:], adj_ap(c),
                                    mybir.AluOpType.mult)
                                emit_epis(1)
                        else:
                            for c in range(NCH):
                                nc.vector.tensor_scalar(
                                    q_t[c][:], w16[h][:], rv_ap(c, h),
                                    rv_ap(c, H + h),
                                    mybir.AluOpType.max, mybir.AluOpType.mult)
                            for c in range(NCH):
                                nc.vector.tensor_tensor(
                                    p_t[c][:], q_t[c][:], adj_ap(c),
                                    mybir.AluOpType.mult)
                                emit_epis(1)
                        emit_epis(len(pending))

                        for c in range(NCH):
                            for ib in range(NCH):
                                nc.tensor.matmul(
                                    acc_ap(ib),
                                    p_t[c][:, ib * P:(ib + 1) * P],
                                    feat2v[c][:, h * EA:(h + 1) * EA],
                                    start=(c == 0 and ib % BSZ == 0),
                                    stop=(c == NCH - 1),
                                    skip_group_check=True,
                                )

                        def make_epi(h, accs):
                            st = {}

                            def acc_ap(ib):
                                b, i = divmod(ib, BSZ)
                                return accs[b][:, i, :]

                            def rec_all():
                                st["rec"] = ep_pool.tile(
                                    [P, NCH], F32, tag="rec", name=f"rec{h}",
                                    padded_shape=[P, 128])
                                for b, bk in enumerate(banks):
                                    nc.vector.reciprocal(
                                        st["rec"][:, bk[0]:bk[0] + len(bk)],
                                        accs[b][:, :, E:E + 1].squeeze(2))
                                HB = max(NCH // 2, 1)
                                st["obh"] = [
                                    ep_pool.tile([P, HB * E], F16,
                                                 tag=f"obh{half}",
                                                 name=f"obh{h}_{half}")
                                    for half in range(2)]

                            def epi_ib(ib, on_dve):
                                HB = max(NCH // 2, 1)
                                ob_ap = st["obh"][ib // HB][
                                    :, (ib % HB) * E:(ib % HB + 1) * E]
                                rec = st["rec"][:, ib:ib + 1]
                                if on_dve:
                                    nc.vector.tensor_scalar(
                                        ob_ap, acc_ap(ib)[:, 0:E], rec, 0.0,
                                        mybir.AluOpType.mult,
                                        mybir.AluOpType.max)
                                else:
                                    nc.scalar.activation(
                                        ob_ap, acc_ap(ib)[:, 0:E],
                                        mybir.ActivationFunctionType.Relu,
                                        scale=rec)

                            def out_dma(half, q, PW, eng):
                                HB = max(NCH // 2, 1)
                                ib0 = half * HB + q * PW
                                eng.dma_start(
                                    out=out[ib0 * P:(ib0 + PW) * P,
                                            h * E:(h + 1) * E].rearrange(
                                        "(ib r) c -> r ib c", r=P),
                                    in_=st["obh"][half][:, q * PW * E:
                                                        (q + 1) * PW * E
                                                        ].rearrange(
                                        "p (ib c) -> p ib c", c=E))
                            return rec_all, epi_ib, out_dma

                        rec_all, epi_ib, out_dma = make_epi(h, accs)
                        if h < H - 1 and DEFER:
                            pending.append(rec_all)
                            for ib in range(NCH):
                                pending.append(
                                    lambda ib=ib: epi_ib(ib, False))
                                if NCH == 8 and ib == 3:
                                    pending.append(
                                        lambda: out_dma(0, 0, 4, nc.sync))
                            if NCH == 8:
                                pending.append(
                                    lambda: out_dma(1, 0, 4, nc.sync))
                            else:
                                HB = max(NCH // 2, 1)
                                for half in range(2):
                                    pending.append(
                                        lambda half=half: out_dma(
                                            half, 0, HB, nc.sync))
                        else:
                            rec_all()
                            for ib in range(NCH):
                                epi_ib(ib, ib % 2 == 1)
                                if NCH == 8:
                                    eng = (nc.sync if ib % 2 == 0
                                           else nc.scalar)
                                    out_dma(ib // 4, ib % 4, 1, eng)
                            if NCH != 8:
                                HB = max(NCH // 2, 1)
                                for half in range(2):
                                    out_dma(half, 0, HB, nc.sync)
                    emit_epis(len(pending))
    nc.compile()
    return nc


_PROGRAM_CACHE = {}


def _get_program(N, H):
    key = (N, H)
    if key not in _PROGRAM_CACHE:
        _PROGRAM_CACHE[key] = build_core_program(N, H)
    return _PROGRAM_CACHE[key]


def host_prep(x, adj, kernel, attn_self, attn_neigh):
    """Per-core input maps: layout transforms, weight folding, and the
    O(N*H) attention row vectors w/r/v (exp transforms of x @ kas/kan)."""
    B, N, D = x.shape
    H, _, E = kernel.shape
    NCH = N // P
    kas = np.empty((D, H), np.float32)
    kan = np.empty((D, H), np.float32)
    kaug = np.empty((D, H * E), np.float32)
    for h in range(H):
        kas[:, h] = kernel[h] @ attn_self[h]
        kan[:, h] = kernel[h] @ attn_neigh[h]
        kaug[:, h * E:(h + 1) * E] = kernel[h]
    in_maps = []
    for b in range(B):
        a_s = x[b] @ kas            # [N, H]
        a_n = x[b] @ kan            # [N, H]
        w4 = np.exp(0.8 * a_s.T).reshape(1, H * N)
        rv = np.concatenate([np.exp(-0.8 * a_n), np.exp(a_n - 2.0)],
                            axis=1)                      # [N, 2H]
        rvT = rv.reshape(NCH, P, 2 * H).transpose(1, 0, 2).reshape(
            P, NCH * 2 * H)
        wx = np.concatenate([kaug, np.ascontiguousarray(x[b].T)], axis=1)
        in_maps.append({
            "wx": np.ascontiguousarray(wx).astype(np.float16),
            "w4": np.ascontiguousarray(w4).astype(np.float16),
            "rvT": np.ascontiguousarray(rvT).astype(np.float32),
            "adjT": np.ascontiguousarray(adj[b].T).astype(np.float16),
        })
    return in_maps


def kernel(x, adj, kernel, attn_self, attn_neigh, bias, _profile=None):
    x = np.asarray(x, np.float32)
    adj = np.asarray(adj, np.float32)
    kernel = np.asarray(kernel, np.float32)
    attn_self = np.asarray(attn_self, np.float32)
    attn_neigh = np.asarray(attn_neigh, np.float32)
    bias = np.asarray(bias, np.float32)

    B, N, D = x.shape
    H, _, E = kernel.shape
    nc = _get_program(N, H)
    in_maps = host_prep(x, adj, kernel, attn_self, attn_neigh)
    kwargs = dict(_profile) if _profile else {}
    last_err = None
    for _attempt in range(3):
        try:
            res = run_bass_kernel_spmd(nc, in_maps, list(range(B)), **kwargs)
            outs = np.stack(
                [np.asarray(res.results[b]["out"]).astype(np.float32)
                 for b in range(B)])
            break
        except Exception as exc:  # transient PJRT/axon fetch errors
            last_err = exc
    else:
        raise last_err
    assert not np.any(bias != 0.0), "nonzero-bias path not implemented"
    if _profile:
        return outs, res
    return outs


if __name__ == "__main__":
    # Mini smoke test: N=256, H=2, B=2 against a numpy reference.
    np.random.seed(0)
    N, H, D, E, B = 256, 2, 128, 128, 2
    LRELU_ALPHA = 0.2
    x = np.random.randn(B, N, D).astype(np.float32)
    adj = (np.random.rand(B, N, N) < 0.5).astype(np.float32)
    K = (np.random.randn(H, D, E) / np.sqrt(D)).astype(np.float32)
    a_s = (np.random.randn(H, E) / np.sqrt(E)).astype(np.float32)
    a_n = (np.random.randn(H, E) / np.sqrt(E)).astype(np.float32)
    bias = np.zeros((H, E), np.float32)

    def ref(x, adj, K, a_s, a_n, bias):
        feat = np.einsum('bnd,hde->bhne', x, K)
        s1 = np.einsum('bhne,he->bhn', feat, a_s)
        s2 = np.einsum('bhne,he->bhn', feat, a_n)
        sc = s1[..., :, None] + s2[..., None, :]
        sc = np.where(sc > 0, sc, LRELU_ALPHA * sc)
        sc = sc + (-1e10) * (1.0 - adj[:, None])
        sc = sc - sc.max(axis=-1, keepdims=True)
        att = np.exp(sc)
        att = att / att.sum(axis=-1, keepdims=True)
        o = np.einsum('bhnm,bhme->bhne', att, feat) + bias[None, :, None, :]
        o = o.transpose(0, 2, 1, 3).reshape(B, N, H * E)
        return np.maximum(o, 0.0)

    expected = ref(x, adj, K, a_s, a_n, bias)
    nc = _get_program(N, H)
    in_maps = host_prep(x, adj, K, a_s, a_n)
    res = run_bass_kernel_spmd(nc, in_maps, list(range(B)))
    actual = np.stack([np.asarray(res.results[b]["out"]).astype(np.float32)
                       for b in range(B)])
    err = np.abs(actual - expected).max() / np.abs(expected).max()
    rel = np.linalg.norm(actual - expected) / np.linalg.norm(expected)
    print(f"SMOKE absmax-rel: {err:.3e}  l2-rel: {rel:.3e}")


# revision 21
# speedup vs baseline: 1.0064x; 1.0064x over previous
"""GAT kernel for Trainium2 (Bass/Tile), data-parallel over batch on 8 cores.

Per-core math (one batch element, N=1024 nodes, H=4 heads, D=E=128). The
softmax numerator exp(lrelu(z_ij)) with z_ij = a_s_i + a_n_j is rewritten
using monotonicity of exp:

  exp(lrelu(z))/s_i = max(w_i, r_j) * v_j       (lrelu = max(z, 0.2 z))
  w = exp(0.8 a_s),  r = exp(-0.8 a_n),  v = exp(a_n - 2),
  s_i = exp(0.2 a_s_i) cancels in the softmax.

a_s/a_n are O(N*H) row vectors; they and their exp transforms are computed
on the host and DMA'd in (~24 KB per core), so the device touches only the
O(N^2) work.  On-core per head:
  p[j,i] = max(w_i, r_j) * adjT[j,i]     (DVE 1-op ts-max + tt-mult; the tt
                                          of 2-3 chunks/head runs on the
                                          otherwise idle GPSIMD)
  num/den accumulate via PE matmuls against [v*feat | v]; v is folded into
  the ACT PSUM->SBUF feature copy via its per-partition scale operand.
PSUM accumulators are bank-packed [P,3,EA]: only the first matmul into a
bank uses start=True (the flag clears has_written for the whole bank), so
eight i-block groups fit in 3 banks and reciprocals batch 3 i-blocks/op.
DMA dispatches are spread across the SP/ACT/GPSIMD queues (each dma_start
occupies its queue ~0.6us) and w-rows ship as one [1,4N] partition-0 strip
so the w16 broadcast (PE ones-outer + DVE/ACT copies) finishes by ~4us."""

import os
import sys

sys.path.insert(0, "/opt/trn_rl_repo")

import numpy as np

import concourse.bass as bass
import concourse.bacc as bacc
import concourse.mybir as mybir
import concourse.tile as tile
from concourse.bass_utils import run_bass_kernel_spmd

F32 = mybir.dt.float32
F16 = mybir.dt.float16
P = 128


def build_core_program(N, H, D=128, E=128):
    """Trace the Bass program computing one batch element of the GAT."""
    nc = bacc.Bacc("TRN2", debug=False, target_bir_lowering=False)
    NCH = N // P          # node chunks
    EA = E + 1            # feat columns + v column (den)
    HP = H // 2
    SEG = 512
    segs = [(s, min(SEG, N - s)) for s in range(0, N, SEG)]
    # i-block banks: groups of <=BSZ share one PSUM bank
    BSZ = int(os.environ.get("GAT_BANKSZ", "3"))
    DEFER = int(os.environ.get("GAT_DEFER", "1"))
    banks = [list(range(b, min(b + BSZ, NCH))) for b in range(0, NCH, BSZ)]

    XOFF = H * E
    wx = nc.dram_tensor("wx", [D, XOFF + N], F16, kind="ExternalInput").ap()
    # per-head w rows
    w4 = nc.dram_tensor("w4", [H, N], F16, kind="ExternalInput").ap()
    # rv pre-transposed on host: [P, NCH * 2H]; cols c*2H+h = r_h, c*2H+H+h = v_h
    rvT = nc.dram_tensor("rvT", [P, NCH * 2 * H], F32,
                         kind="ExternalInput").ap()
    adjT = nc.dram_tensor("adjT", [N, N], F16, kind="ExternalInput").ap()
    out = nc.dram_tensor("out", [N, H * E], F16, kind="ExternalOutput").ap()

    # gpsimd tt chunks per head (late chunks; none for the last head's tail)
    GPS_DEF = ",".join([""] * H)
    gps_spec = os.environ.get("GAT_GPSTT", GPS_DEF).split(",")
    GPS_CHUNKS = [set(int(ch) for ch in s) if s else set()
                  for s in (gps_spec + [""] * H)[:H]]
    XSEG = min(int(os.environ.get("GAT_XSEG", "256")), N)
    ASEG0 = min(int(os.environ.get("GAT_ASEG0", "256")), N)
    ASEG = min(int(os.environ.get("GAT_ASEG", "512")), N)

    with tile.TileContext(nc) as tc:
        with (
            tc.tile_pool(name="const", bufs=1) as const_pool,
            tc.tile_pool(name="xt", bufs=1) as xt_pool,
            tc.tile_pool(name="adj", bufs=1) as adj_pool,
            tc.tile_pool(name="fr", bufs=1) as fr_pool,
            tc.tile_pool(name="rv", bufs=1) as rv_pool,
            tc.tile_pool(name="wr", bufs=1) as wr_pool,
            tc.tile_pool(name="w16", bufs=1) as w16_pool,
        ):
            ones_sb = const_pool.tile([1, P], F16, tag="ones", padded_shape=[1, 256])
            nc.vector.memset(ones_sb[:], 1.0)

            kaug_t = xt_pool.tile([D, H * E], F16, tag="kaug")
            NXT = max(N // 512, 1)
            xt_t = [xt_pool.tile([D, min(512, N)], F16, tag=f"xt{i}",
                                 name=f"xt{i}") for i in range(NXT)]
            w4row = [wr_pool.tile([1, N], F16, tag=f"w4r{h}",
                                  name=f"w4r{h}") for h in range(H)]
            RVPAD = ((NCH * 2 * H + 127) // 128) * 128
            rv2f = rv_pool.tile([P, NCH * 2 * H], F32, tag="rv2",
                                padded_shape=[P, RVPAD])

            def rv_ap(c, idx):
                return rv2f[:, c * 2 * H + idx:c * 2 * H + idx + 1]

            def xt_cols(s0, w):
                ti, off = s0 // 512, s0 % 512
                assert off + w <= 512
                return xt_t[ti][:, off:off + w]

            QC = 4 if NCH % 4 == 0 else 1
            NQ = NCH // QC
            adj_sb = [adj_pool.tile([P, QC * N], F16, tag=f"adj{c4}",
                                    name=f"adj{c4}") for c4 in range(NQ)]

            def adj_ap(c):
                c4, kq = c // QC, c % QC
                return adj_sb[c4][:, kq * N:(kq + 1) * N]

            def adj_dma(eng, c, aseg):
                for s in range(0, N, aseg):
                    eng.dma_start(out=adj_ap(c)[:, s:s + aseg],
                                  in_=adjT[c * P:(c + 1) * P, s:s + aseg])

            with tc.high_priority():
                # scalar (HWDGE): rv + per-head w rows first (gate the mask)
                nc.scalar.dma_start(out=rv2f[:], in_=rvT)
                for h in range(H):
                    nc.scalar.dma_start(out=w4row[h][:], in_=w4[h:h + 1, :])
                adj_dma(nc.sync, 0, ASEG0)
                if NCH > 2:
                    adj_dma(nc.sync, 2, ASEG)
                for s in range(0, H * E, 256):
                    nc.sync.dma_start(out=kaug_t[:, s:s + 256],
                                      in_=wx[:, s:s + 256])
                if NCH > 4:
                    adj_dma(nc.sync, 4, ASEG)
                xslices = ([128, 128, 256, 256, 256] if N == 1024
                           else [XSEG] * (N // XSEG))
                xq = []
                s = 0
                for sw in xslices:
                    ti, off = s // 512, s % 512
                    xq.append((xt_t[ti], off, sw, s))
                    s += sw
                for i, (t, off, sw, s0) in enumerate(xq):
                    nc.sync.dma_start(out=t[:, off:off + sw],
                                      in_=wx[:, XOFF + s0:XOFF + s0 + sw])
                    if i == 1 and NCH > 6:
                        adj_dma(nc.sync, 6, ASEG)
                # scalar (HWDGE): odd adj chunks (its copies start later)
                for c in range(1, NCH, 2):
                    adj_dma(nc.scalar, c, ASEG)

            w16 = [w16_pool.tile([P, N], F16, tag=f"w16_{h}", name=f"w16_{h}")
                   for h in range(H)]
            FRPAD = ((H * EA + 255) // 256) * 256
            feat2v = [fr_pool.tile([P, H * EA], F16, tag=f"fr{c}",
                                   name=f"fr{c}", padded_shape=[P, FRPAD])
                      for c in range(NCH)]

            with (
                tc.tile_pool(name="acc_e", bufs=1, space="PSUM") as acc_e,
                tc.tile_pool(name="acc_o", bufs=1, space="PSUM") as acc_o,
                tc.tile_pool(name="proj_ps", bufs=2, space="PSUM") as proj_ps,
            ):
                # ---- w16 broadcast: PE ones-outer -> ACT copy via
                # borrowed acc_o banks (gpsimd stays off the SBUF port)
                bc_tags = [f"b{i}" for i in range(len(banks))]
                bc_i = 0

                def bcast_w16(h):
                    nonlocal bc_i
                    for s0, sw in segs:
                        tg = bc_tags[bc_i % len(bc_tags)]
                        bc_i += 1
                        bc = acc_o.tile([P, max(SEG, BSZ * EA)], F32, tag=tg,
                                        name=f"bc{h}_{s0}")
                        nc.tensor.matmul(
                            bc[:, 0:sw], ones_sb[:],
                            w4row[h][:, s0:s0 + sw],
                            start=True, stop=True)
                        nc.scalar.copy(w16[h][:, s0:s0 + sw], bc[:, 0:sw])

                with tc.high_priority():
                    bcast_w16(0)
                for h in range(1, H):
                    bcast_w16(h)

                # ---- projection + v-scaled feature copies + v columns
                for c in range(NCH):
                    f3 = feat2v[c][:].rearrange("p (h f) -> p h f", h=H)
                    # den columns are plain ones (v lives in the ts output)
                    nc.vector.memset(f3[:, :, E:E + 1].squeeze(2), 1.0)
                    for hp in range(HP):
                        ps = proj_ps.tile([P, 2 * E], F32, tag="proj",
                                          name=f"proj{hp}_{c}")
                        nc.tensor.matmul(
                            ps[:], xt_cols(c * P, P),
                            kaug_t[:, hp * 2 * E:(hp + 1) * 2 * E],
                            start=True, stop=True)
                        ps3 = ps[:].rearrange("p (k f) -> p k f", k=2)
                        nc.scalar.copy(f3[:, 2 * hp:2 * hp + 2, 0:E], ps3[:])

                with (
                    tc.tile_pool(name="q", bufs=2) as q_pool,
                    tc.tile_pool(name="ep", bufs=2) as ep_pool,
                ):
                    pending = []

                    def emit_epis(k):
                        for _ in range(k):
                            if pending:
                                pending.pop(0)()

                    for h in range(H):
                        acc_pool = acc_e if h % 2 == 0 else acc_o
                        tags = (["A", "B", "C"] if h % 2 == 0 else bc_tags)
                        accs = [acc_pool.tile([P, len(bk), EA], F32,
                                              tag=tags[b % len(tags)],
                                              name=f"acc{h}_{b}")
                                for b, bk in enumerate(banks)]

                        def acc_ap(ib):
                            b, i = divmod(ib, BSZ)
                            return accs[b][:, i, :]

                        # masks: all ts first (adj-tolerant), then tts
                        q_t = [q_pool.tile([P, N], F16, tag=f"q{c}",
                                           name=f"q{h}_{c}")
                               for c in range(NCH)]
                        p_t = [q_pool.tile([P, N], F16, tag=f"p{c}",
                                           name=f"p{h}_{c}")
                               for c in range(NCH)]
                        ILV = int(os.environ.get("GAT_ILV", "0"))
                        if ILV:
                            for c in range(NCH):
                                nc.vector.tensor_scalar(
                                    q_t[c][:], w16[h][:], rv_ap(c, h),
                                    rv_ap(c, H + h),
                                    mybir.AluOpType.max, mybir.AluOpType.mult)
                                nc.vector.tensor_tensor(
                                    p_t[c][:], q_t[c][:], adj_ap(c),
                                    mybir.AluOpType.mult)
                                emit_epis(1)
                        else:
                            for c in range(NCH):
                                nc.vector.tensor_scalar(
                                    q_t[c][:], w16[h][:], rv_ap(c, h),
                                    rv_ap(c, H + h),
                                    mybir.AluOpType.max, mybir.AluOpType.mult)
                            for c in range(NCH):
                                nc.vector.tensor_tensor(
                                    p_t[c][:], q_t[c][:], adj_ap(c),
                                    mybir.AluOpType.mult)
                                emit_epis(1)
                        emit_epis(len(pending))

                        for c in range(NCH):
                            for ib in range(NCH):
                                nc.tensor.matmul(
                                    acc_ap(ib),
                                    p_t[c][:, ib * P:(ib + 1) * P],
                                    feat2v[c][:, h * EA:(h + 1) * EA],
                                    start=(c == 0 and ib % BSZ == 0),
                                    stop=(c == NCH - 1),
                                    skip_group_check=True,
                                )

                        def make_epi(h, accs):
                            st = {}

                            def acc_ap(ib):
                                b, i = divmod(ib, BSZ)
                                return accs[b][:, i, :]

                            def rec_all():
                                st["rec"] = ep_pool.tile(
                                    [P, NCH], F32, tag="rec", name=f"rec{h}",
                                    padded_shape=[P, 128])
                                for b, bk in enumerate(banks):
                                    nc.vector.reciprocal(
                                        st["rec"][:, bk[0]:bk[0] + len(bk)],
                                        accs[b][:, :, E:E + 1].squeeze(2))
                                HB = max(NCH // 2, 1)
                                st["obh"] = [
                                    ep_pool.tile([P, HB * E], F16,
                                                 tag=f"obh{half}",
                                                 name=f"obh{h}_{half}")
                                    for half in range(2)]

                            def epi_ib(ib, on_dve):
                                HB = max(NCH // 2, 1)
                                ob_ap = st["obh"][ib // HB][
                                    :, (ib % HB) * E:(ib % HB + 1) * E]
                                rec = st["rec"][:, ib:ib + 1]
                                if on_dve:
                                    nc.vector.tensor_scalar(
                                        ob_ap, acc_ap(ib)[:, 0:E], rec, 0.0,
                                        mybir.AluOpType.mult,
                                        mybir.AluOpType.max)
                                else:
                                    nc.scalar.activation(
                                        ob_ap, acc_ap(ib)[:, 0:E],
                                        mybir.ActivationFunctionType.Relu,
                                        scale=rec)

                            def out_dma(half, q, PW, eng):
                                HB = max(NCH // 2, 1)
                                ib0 = half * HB + q * PW
                                eng.dma_start(
                                    out=out[ib0 * P:(ib0 + PW) * P,
                                            h * E:(h + 1) * E].rearrange(
                                        "(ib r) c -> r ib c", r=P),
                                    in_=st["obh"][half][:, q * PW * E:
                                                        (q + 1) * PW * E
                                                        ].rearrange(
                                        "p (ib c) -> p ib c", c=E))
                            return rec_all, epi_ib, out_dma

                        rec_all, epi_ib, out_dma = make_epi(h, accs)
                        if h < H - 1 and DEFER:
                            pending.append(rec_all)
                            for ib in range(NCH):
                                pending.append(
                                    lambda ib=ib: epi_ib(ib, False))
                                if NCH == 8 and ib == 3:
                                    pending.append(
                                        lambda: out_dma(0, 0, 4, nc.sync))
                            if NCH == 8:
                                pending.append(
                                    lambda: out_dma(1, 0, 4, nc.sync))
                            else:
                                HB = max(NCH // 2, 1)
                                for half in range(2):
                                    pending.append(
                                        lambda half=half: out_dma(
                                            half, 0, HB, nc.sync))
                        else:
                            rec_all()
                            for ib in range(NCH):
                                epi_ib(ib, ib % 2 == 1)
                                if NCH == 8:
                                    eng = (nc.sync if ib % 2 == 0
                                           else nc.scalar)
                                    out_dma(ib // 4, ib % 4, 1, eng)
                            if NCH != 8:
                                HB = max(NCH // 2, 1)
                                for half in range(2):
                                    out_dma(half, 0, HB, nc.sync)
                    emit_epis(len(pending))
    nc.compile()
    return nc


_PROGRAM_CACHE = {}


def _get_program(N, H):
    key = (N, H)
    if key not in _PROGRAM_CACHE:
        _PROGRAM_CACHE[key] = build_core_program(N, H)
    return _PROGRAM_CACHE[key]


def host_prep(x, adj, kernel, attn_self, attn_neigh):
    """Per-core input maps: layout transforms, weight folding, and the
    O(N*H) attention row vectors w/r/v (exp transforms of x @ kas/kan)."""
    B, N, D = x.shape
    H, _, E = kernel.shape
    NCH = N // P
    kas = np.empty((D, H), np.float32)
    kan = np.empty((D, H), np.float32)
    kaug = np.empty((D, H * E), np.float32)
    for h in range(H):
        kas[:, h] = kernel[h] @ attn_self[h]
        kan[:, h] = kernel[h] @ attn_neigh[h]
        kaug[:, h * E:(h + 1) * E] = kernel[h]
    in_maps = []
    for b in range(B):
        a_s = x[b] @ kas            # [N, H]
        a_n = x[b] @ kan            # [N, H]
        w4 = np.exp(0.8 * a_s.T)
        rv = np.concatenate([np.exp(-0.8 * a_n), np.exp(a_n - 2.0)],
                            axis=1)                      # [N, 2H]
        rvT = rv.reshape(NCH, P, 2 * H).transpose(1, 0, 2).reshape(
            P, NCH * 2 * H)
        wx = np.concatenate([kaug, np.ascontiguousarray(x[b].T)], axis=1)
        in_maps.append({
            "wx": np.ascontiguousarray(wx).astype(np.float16),
            "w4": np.ascontiguousarray(w4).astype(np.float16),
            "rvT": np.ascontiguousarray(rvT).astype(np.float32),
            "adjT": np.ascontiguousarray(adj[b].T).astype(np.float16),
        })
    return in_maps


def kernel(x, adj, kernel, attn_self, attn_neigh, bias, _profile=None):
    x = np.asarray(x, np.float32)
    adj = np.asarray(adj, np.float32)
    kernel = np.asarray(kernel, np.float32)
    attn_self = np.asarray(attn_self, np.float32)
    attn_neigh = np.asarray(attn_neigh, np.float32)
    bias = np.asarray(bias, np.float32)

    B, N, D = x.shape
    H, _, E = kernel.shape
    nc = _get_program(N, H)
    in_maps = host_prep(x, adj, kernel, attn_self, attn_neigh)
    kwargs = dict(_profile) if _profile else {}
    last_err = None
    for _attempt in range(3):
        try:
            res = run_bass_kernel_spmd(nc, in_maps, list(range(B)), **kwargs)
            outs = np.stack(
                [np.asarray(res.results[b]["out"]).astype(np.float32)
                 for b in range(B)])
            break
        except Exception as exc:  # transient PJRT/axon fetch errors
            last_err = exc
    else:
        raise last_err
    assert not np.any(bias != 0.0), "nonzero-bias path not implemented"
    if _profile:
        return outs, res
    return outs


if __name__ == "__main__":
    # Mini smoke test: N=256, H=2, B=2 against a numpy reference.
    np.random.seed(0)
    N, H, D, E, B = 256, 2, 128, 128, 2
    LRELU_ALPHA = 0.2
    x = np.random.randn(B, N, D).astype(np.float32)
    adj = (np.random.rand(B, N, N) < 0.5).astype(np.float32)
    K = (np.random.randn(H, D, E) / np.sqrt(D)).astype(np.float32)
    a_s = (np.random.randn(H, E) / np.sqrt(E)).astype(np.float32)
    a_n = (np.random.randn(H, E) / np.sqrt(E)).astype(np.float32)
    bias = np.zeros((H, E), np.float32)

    def ref(x, adj, K, a_s, a_n, bias):
        feat = np.einsum('bnd,hde->bhne', x, K)
        s1 = np.einsum('bhne,he->bhn', feat, a_s)
        s2 = np.einsum('bhne,he->bhn', feat, a_n)
        sc = s1[..., :, None] + s2[..., None, :]
        sc = np.where(sc > 0, sc, LRELU_ALPHA * sc)
        sc = sc + (-1e10) * (1.0 - adj[:, None])
        sc = sc - sc.max(axis=-1, keepdims=True)
        att = np.exp(sc)
        att = att / att.sum(axis=-1, keepdims=True)
        o = np.einsum('bhnm,bhme->bhne', att, feat) + bias[None, :, None, :]
        o = o.transpose(0, 2, 1, 3).reshape(B, N, H * E)
        return np.maximum(o, 0.0)

    expected = ref(x, adj, K, a_s, a_n, bias)
    nc = _get_program(N, H)
    in_maps = host_prep(x, adj, K, a_s, a_n)
    res = run_bass_kernel_spmd(nc, in_maps, list(range(B)))
    actual = np.stack([np.asarray(res.results[b]["out"]).astype(np.float32)
                       for b in range(B)])
    err = np.abs(actual - expected).max() / np.abs(expected).max()
    rel = np.linalg.norm(actual - expected) / np.linalg.norm(expected)
    print(f"SMOKE absmax-rel: {err:.3e}  l2-rel: {rel:.3e}")
